# revision 1
# baseline (speedup 1.0000x reference)
"""Trainium2 Bass kernel for the fused 3-modality attention + FFN + softmax model.

Layout strategy: pure data parallel over 8 NeuronCores (batch sharded), all
activations kept FEATURE-MAJOR on chip ([1024 feats = 8 chunks x 128
partitions, tokens in the free dim]) so no on-device transposes are needed.
All GEMMs run in bf16 (1 cycle/row on the PE) accumulating fp32 in PSUM.
LayerNorm reductions over features are ones-vector matmuls on the PE;
per-token scalars are broadcast back across partitions with K=1 expand
matmuls. Host-side prep: transpose activations to [DIM, B] bf16, pre-scale
Wg by 1/3 (modality mean) and Wq/bq by 1/sqrt(HD) (attention scale).
"""

import numpy as np
import ml_dtypes

import concourse.bacc as bacc
import concourse.bass as bass
import concourse.mybir as mybir
import concourse.tile as tile

B, DIM, H, FFN, HD = 16384, 1024, 16, 4096, 64
NCORES = 8
TPC = B // NCORES          # tokens per core
TB = 512                   # token block (matmul moving dim)
KC = DIM // 128            # 8 feature chunks
MC1 = FFN // 128           # 32 ffn chunks
EPS = 1e-5

BF16 = mybir.dt.bfloat16
F32 = mybir.dt.float32
F32R = mybir.dt.float32r
AF = mybir.ActivationFunctionType


def _ln_apply(nc, pp, wk, src_f32, out_bf, g, be, C, cbufs=2):
    """LayerNorm over features (partition x chunk axis); src modified in place.

    src_f32: [128, KC*TB] fp32 tile, out_bf: [128, KC*TB] bf16 tile.
    Per-token scalars live as rows of one packed [8, TB] fp32 tile.
    """
    v, s, te = nc.vector, nc.scalar, nc.tensor
    # bf16 copy of src for the (cheap, 1 cyc/row) column-sum matmuls
    xbc = wk.tile([128, KC * TB], BF16, tag="a4", bufs=cbufs, name="xbc")
    s.activation(xbc[:], src_f32[:], AF.Copy)
    pr1 = pp.tile([16, TB], F32, tag="red", bufs=3, name="pr1")
    for kc in range(KC):
        te.matmul(pr1[0:1, :], C["onecb"][:],
                  xbc[:, kc * TB:(kc + 1) * TB],
                  start=(kc == 0), stop=(kc == KC - 1))
    sq = wk.tile([128, KC * TB], BF16, tag="qb", bufs=1, name="sq")
    s.activation(sq[:], src_f32[:], AF.Square)
    pr2 = pp.tile([16, TB], F32, tag="red", bufs=3, name="pr2")
    for kc in range(KC):
        te.matmul(pr2[0:1, :], C["onecb"][:], sq[:, kc * TB:(kc + 1) * TB],
                  start=(kc == 0), stop=(kc == KC - 1))
    # per-token scalars: separate base-0 tiles (partition-alignment rules)
    mub = wk.tile([1, TB], BF16, tag="ln_mub", bufs=1, name="mub")[:]
    ex2 = wk.tile([1, TB], F32, tag="ln_ex2", bufs=1, name="ex2")[:]
    mu2 = wk.tile([1, TB], F32, tag="ln_mu2", bufs=1, name="mu2")[:]
    var = wk.tile([1, TB], F32, tag="ln_var", bufs=1, name="var")[:]
    sd = wk.tile([1, TB], F32, tag="ln_sd", bufs=1, name="sd")[:]
    rs = wk.tile([1, TB], F32, tag="ln_rs", bufs=1, name="rs")[:]
    rsb = wk.tile([1, TB], BF16, tag="ln_rsb", bufs=1, name="rsb")[:]
    s.activation(mub, pr1[0:1, :], AF.Copy, scale=1.0 / DIM)
    s.activation(ex2, pr2[0:1, :], AF.Copy, scale=1.0 / DIM)
    s.activation(mu2, mub, AF.Square)
    v.tensor_sub(var, ex2, mu2)
    s.activation(sd, var, AF.Sqrt, bias=C["epsc"][:])
    v.reciprocal(rs, sd)
    s.activation(rsb, rs, AF.Copy)
    pmu = pp.tile([128, TB], F32, tag="acc", bufs=5, name="pmu")
    te.matmul(pmu[:], C["onerb"][:], mub, start=True, stop=True)
    prs = pp.tile([128, TB], F32, tag="acc", bufs=5, name="prs")
    te.matmul(prs[:], C["onerb"][:], rsb, start=True, stop=True)
    for kc in range(KC):
        sl = src_f32[:, kc * TB:(kc + 1) * TB]
        v.tensor_sub(sl, sl, pmu[:])
        v.tensor_mul(sl, sl, prs[:])
        s.activation(out_bf[:, kc * TB:(kc + 1) * TB], sl, AF.Identity,
                     scale=g[:, kc:kc + 1], bias=be[:, kc:kc + 1])


def _emit(nc, tc, io, tpc):
    nblk = tpc // TB
    v, s, te = nc.vector, nc.scalar, nc.tensor

    with (
        tc.tile_pool(name="consts", bufs=1) as cp,
        tc.tile_pool(name="psum", bufs=1, space="PSUM") as pp,
    ):
        # ---- constants / small params ----
        C = {}
        for name, shape, dtype in (
            ("Ssel", [128, 128], BF16), ("Eexp", [16, 1024], BF16),
            ("onecb", [128, 1], BF16), ("onerb", [1, 128], BF16),
            ("bg", [128, KC], F32), ("bq", [128, KC], F32),
            ("bk", [128, KC], F32), ("bv", [128, KC], F32),
            ("b1", [128, MC1], F32), ("b2", [128, KC], F32),
            ("g1", [128, KC], F32), ("be1", [128, KC], F32),
            ("g2", [128, KC], F32), ("be2", [128, KC], F32),
            ("Wwt", [128, 3 * KC], BF16), ("bwc", [1, 3], F32),
            ("epsc", [1, 1], F32),
        ):
            t = cp.tile(shape, dtype, name=f"c_{name}")
            nc.sync.dma_start(out=t[:], in_=io[name])
            C[name] = t

        # ---------------- phase A: attention + LN1 ----------------
        with (
            tc.tile_pool(name="wA", bufs=1) as wa,
            tc.tile_pool(name="workA", bufs=1) as wk,
        ):
            wmap = {}
            for wn in ("Wg", "Wq", "Wk", "Wv"):
                wt = wa.tile([128, KC * DIM], BF16, name=f"w_{wn}")
                nc.sync.dma_start(
                    out=wt[:].rearrange("p (c n) -> p c n", n=DIM),
                    in_=io[wn].rearrange("(c p) n -> p c n", p=128))
                wmap[wn] = wt

            def wsl(wn, kc, mc):
                return wmap[wn][:, kc * DIM + mc * 128:kc * DIM + mc * 128 + 128]

            for blk in range(nblk):
                t0 = blk * TB
                ins = {}
                for name in ("m0", "m1", "m2", "dom"):
                    t = wk.tile([128, KC * TB], BF16, tag=f"in_{name}",
                                bufs=(2 if name == "dom" else 1),
                                name=f"{name}_sb")
                    nc.sync.dma_start(
                        out=t[:].rearrange("p (c t) -> p c t", t=TB),
                        in_=io[name].rearrange("(c p) t -> p c t",
                                               p=128)[:, :, t0:t0 + TB])
                    ins[name] = t
                mj = [ins["m0"], ins["m1"], ins["m2"]]
                dom = ins["dom"]

                avg = wk.tile([128, KC * TB], BF16, tag="a4", bufs=2, name="avg")
                v.tensor_add(avg[:], mj[0][:], mj[1][:])
                v.tensor_add(avg[:], avg[:], mj[2][:])

                # global_rep -> qin: (psum + bg) + dom fused on the DVE
                qin = wk.tile([128, KC * TB], BF16, tag="a4", bufs=2, name="qin")
                for mc in range(KC):
                    pg = pp.tile([128, TB], F32, tag="acc", bufs=5, name="pg")
                    for kc in range(KC):
                        te.matmul(pg[:], wsl("Wg", kc, mc),
                                  avg[:, kc * TB:(kc + 1) * TB],
                                  start=(kc == 0), stop=(kc == KC - 1))
                    v.scalar_tensor_tensor(
                        qin[:, mc * TB:(mc + 1) * TB], pg[:],
                        C["bg"][:, mc:mc + 1], dom[:, mc * TB:(mc + 1) * TB],
                        mybir.AluOpType.add, mybir.AluOpType.add)

                qb = wk.tile([128, KC * TB], BF16, tag="qb", bufs=1, name="qb")
                for mc in range(KC):
                    pq = pp.tile([128, TB], F32, tag="acc", bufs=5, name="pq")
                    for kc in range(KC):
                        te.matmul(pq[:], wsl("Wq", kc, mc),
                                  qin[:, kc * TB:(kc + 1) * TB],
                                  start=(kc == 0), stop=(kc == KC - 1))
                    s.activation(qb[:, mc * TB:(mc + 1) * TB], pq[:],
                                 AF.Identity, bias=C["bq"][:, mc:mc + 1])

                # scores[h,t] per modality (Wq/bq pre-scaled by 1/sqrt(HD))
                sc = wk.tile([16, 3 * TB], F32, tag="sc", bufs=1,
                             name="sc")
                for mc in range(KC):
                    pks = []
                    for j in range(3):
                        pks.append(pp.tile([128, TB], F32, tag="acc", bufs=5,
                                           name=f"pk{j}"))
                    for kc in range(KC):
                        for j in range(3):
                            te.matmul(pks[j][:], wsl("Wk", kc, mc),
                                      mj[j][:, kc * TB:(kc + 1) * TB],
                                      start=(kc == 0), stop=(kc == KC - 1))
                    for j in range(3):
                        tm = wk.tile([128, TB], BF16, tag="tm", bufs=2,
                                     name="tm")
                        v.scalar_tensor_tensor(
                            tm[:], pks[j][:], C["bk"][:, mc:mc + 1],
                            qb[:, mc * TB:(mc + 1) * TB],
                            mybir.AluOpType.add, mybir.AluOpType.mult)
                        ps = pp.tile([16, TB], F32, tag="red", bufs=3,
                                     name="ps")
                        te.matmul(ps[:], C["Ssel"][:, mc * 16:(mc + 1) * 16],
                                  tm[:], start=True, stop=True)
                        scj = sc[:, j * TB:(j + 1) * TB]
                        if mc == 0:
                            v.tensor_copy(scj, ps[:])
                        else:
                            v.tensor_add(scj, scj, ps[:])

                # softmax over the 3 modalities (all tiles at base 0)
                mx = wk.tile([16, TB], F32, tag="mx", bufs=1, name="mx")[:]
                sm = wk.tile([16, TB], F32, tag="sm", bufs=1, name="sm")[:]
                rc = wk.tile([16, TB], F32, tag="rc", bufs=1, name="rc")[:]
                v.tensor_max(mx, sc[:, 0:TB], sc[:, TB:2 * TB])
                v.tensor_max(mx, mx, sc[:, 2 * TB:3 * TB])
                for j in range(3):
                    scj = sc[:, j * TB:(j + 1) * TB]
                    v.tensor_sub(scj, scj, mx)
                ab = wk.tile([16, 3 * TB], BF16, tag="ab", bufs=1, name="ab")
                s.activation(ab[:], sc[:], AF.Exp)
                v.tensor_add(sm, ab[:, 0:TB], ab[:, TB:2 * TB])
                v.tensor_add(sm, sm, ab[:, 2 * TB:3 * TB])
                v.reciprocal(rc, sm)
                for j in range(3):
                    abj = ab[:, j * TB:(j + 1) * TB]
                    v.tensor_mul(abj, abj, rc)

                # attnout = sum_j bcast(attn_j) * (m_j @ Wv); bv folds to
                # +bv since sum_j attn_j = 1. All 24 v-matmuls for a chunk
                # are emitted before the attn-dependent expands so the PE
                # never stalls waiting for the softmax.
                xp = wk.tile([128, KC * TB], F32, tag="xp", bufs=1, name="xp")
                for mc in range(KC):
                    pvs = []
                    for j in range(3):
                        pvs.append(pp.tile([128, TB], F32, tag="acc", bufs=5,
                                           name=f"pv{j}"))
                    for kc in range(KC):
                        for j in range(3):
                            te.matmul(pvs[j][:], wsl("Wv", kc, mc),
                                      mj[j][:, kc * TB:(kc + 1) * TB],
                                      start=(kc == 0), stop=(kc == KC - 1))
                    acc = wk.tile([128, TB], F32, tag="acc_s", bufs=2,
                                  name="acc")
                    for j in range(3):
                        vt = wk.tile([128, TB], BF16, tag="vt", bufs=2,
                                     name="vt")
                        s.activation(vt[:], pvs[j][:], AF.Copy)
                        pa = pp.tile([128, TB], F32, tag="red", bufs=3,
                                     name="pa")
                        te.matmul(pa[:], C["Eexp"][:, mc * 128:(mc + 1) * 128],
                                  ab[:, j * TB:(j + 1) * TB],
                                  start=True, stop=True)
                        if j == 0:
                            v.tensor_mul(acc[:], pa[:], vt[:])
                        else:
                            t2 = wk.tile([128, TB], F32, tag="t2", bufs=2,
                                         name="t2")
                            v.tensor_mul(t2[:], pa[:], vt[:])
                            v.tensor_add(acc[:], acc[:], t2[:])
                    v.scalar_tensor_tensor(
                        xp[:, mc * TB:(mc + 1) * TB], acc[:],
                        C["bv"][:, mc:mc + 1], dom[:, mc * TB:(mc + 1) * TB],
                        mybir.AluOpType.add, mybir.AluOpType.add)

                x_bf = wk.tile([128, KC * TB], BF16, tag="xbf", bufs=2,
                               name="x_bf")
                _ln_apply(nc, pp, wk, xp, x_bf, C["g1"], C["be1"], C)
                nc.sync.dma_start(
                    out=io["xs"].rearrange("(c p) t -> p c t",
                                           p=128)[:, :, t0:t0 + TB],
                    in_=x_bf[:].rearrange("p (c t) -> p c t", t=TB))

        # ---------------- phase B: FFN + LN2 + logits ----------------
        with (
            tc.tile_pool(name="wB", bufs=1) as wb,
            tc.tile_pool(name="workB", bufs=1) as wk,
        ):
            w1k = []
            for kc in range(KC):
                t = wb.tile([128, FFN], BF16, name=f"w_W1_{kc}")
                nc.sync.dma_start(
                    out=t[:],
                    in_=io["W1"].rearrange("(c p) n -> p c n",
                                           p=128)[:, kc, :])
                w1k.append(t)

            for blk in range(nblk):
                t0 = blk * TB
                xb = wk.tile([128, KC * TB], BF16, tag="xb", bufs=1, name="xb")
                nc.sync.dma_start(
                    out=xb[:].rearrange("p (c t) -> p c t", t=TB),
                    in_=io["xs"].rearrange("(c p) t -> p c t",
                                           p=128)[:, :, t0:t0 + TB])
                hb = wk.tile([128, MC1 * TB], BF16, tag="hb", bufs=1, name="hb")
                for mc in range(MC1):
                    ph = pp.tile([128, TB], F32, tag="acc", bufs=5, name="ph")
                    for kc in range(KC):
                        te.matmul(ph[:],
                                  w1k[kc][:, mc * 128:mc * 128 + 128],
                                  xb[:, kc * TB:(kc + 1) * TB],
                                  start=(kc == 0), stop=(kc == KC - 1))
                    s.activation(hb[:, mc * TB:(mc + 1) * TB], ph[:], AF.Relu,
                                 bias=C["b1"][:, mc:mc + 1])

                x2 = wk.tile([128, KC * TB], F32, tag="x2", bufs=1, name="x2")
                for mc in range(KC):
                    w2t = wk.tile([128, MC1 * 128], BF16, tag="w2t", bufs=2,
                                  name="w2t")
                    nc.sync.dma_start(
                        out=w2t[:].rearrange("p (c n) -> p c n", n=128),
                        in_=io["W2"].rearrange("(c p) n -> p c n",
                                               p=128)[:, :,
                                                      mc * 128:(mc + 1) * 128])
                    pf = pp.tile([128, TB], F32, tag="acc", bufs=5, name="pf")
                    for kc in range(MC1):
                        te.matmul(pf[:], w2t[:, kc * 128:(kc + 1) * 128],
                                  hb[:, kc * TB:(kc + 1) * TB],
                                  start=(kc == 0), stop=(kc == MC1 - 1))
                    v.scalar_tensor_tensor(
                        x2[:, mc * TB:(mc + 1) * TB], pf[:],
                        C["b2"][:, mc:mc + 1], xb[:, mc * TB:(mc + 1) * TB],
                        mybir.AluOpType.add, mybir.AluOpType.add)

                yb = wk.tile([128, KC * TB], BF16, tag="yb", bufs=1, name="yb")
                _ln_apply(nc, pp, wk, x2, yb, C["g2"], C["be2"], C, cbufs=1)

                # logits: one single-row matmul accumulation per class so
                # every scalar row lives at partition base 0.
                zc, ec = [], []
                for c in range(3):
                    pzc = pp.tile([1, TB], F32, tag="red", bufs=3,
                                  name=f"pz{c}")
                    for kc in range(KC):
                        te.matmul(pzc[:],
                                  C["Wwt"][:, kc * 3 + c:kc * 3 + c + 1],
                                  yb[:, kc * TB:(kc + 1) * TB],
                                  start=(kc == 0), stop=(kc == KC - 1))
                    zt = wk.tile([1, TB], F32, tag=f"z{c}", bufs=1,
                                 name=f"z{c}")
                    s.activation(zt[:], pzc[:], AF.Identity,
                                 bias=C["bwc"][:, c:c + 1])
                    zc.append(zt[:])
                mx3 = wk.tile([1, TB], F32, tag="mx3", bufs=1, name="mx3")[:]
                ss = wk.tile([1, TB], F32, tag="ss", bufs=1, name="ss")[:]
                rr = wk.tile([1, TB], F32, tag="rr", bufs=1, name="rr")[:]
                v.tensor_max(mx3, zc[0], zc[1])
                v.tensor_max(mx3, mx3, zc[2])
                for c in range(3):
                    et = wk.tile([1, TB], F32, tag=f"e{c}", bufs=1,
                                 name=f"e{c}")
                    v.tensor_sub(et[:], zc[c], mx3)
                    s.activation(et[:], et[:], AF.Exp)
                    ec.append(et[:])
                v.tensor_add(ss, ec[0], ec[1])
                v.tensor_add(ss, ss, ec[2])
                v.reciprocal(rr, ss)
                for c in range(3):
                    pt = wk.tile([1, TB], F32, tag=f"p{c}", bufs=1,
                                 name=f"p{c}")
                    v.tensor_mul(pt[:], ec[c], rr)
                    nc.sync.dma_start(
                        out=io["out"][t0:t0 + TB, c:c + 1].rearrange(
                            "t c -> c t"),
                        in_=pt[:])


def build_program(tpc=TPC):
    nc = bacc.Bacc("TRN2", target_bir_lowering=False, debug=False)
    io = {}

    def din(name, shape, dtype):
        io[name] = nc.dram_tensor(name, shape, dtype, kind="ExternalInput").ap()

    for name in ("m0", "m1", "m2", "dom"):
        din(name, [DIM, tpc], BF16)
    for name in ("Wg", "Wq", "Wk", "Wv"):
        din(name, [DIM, DIM], BF16)
    din("W1", [DIM, FFN], BF16)
    din("W2", [FFN, DIM], BF16)
    din("Ssel", [128, 128], BF16)
    din("Eexp", [16, 1024], BF16)
    din("onecb", [128, 1], BF16)
    din("onerb", [1, 128], BF16)
    for name, w in (("bg", KC), ("bq", KC), ("bk", KC), ("bv", KC),
                    ("b1", MC1), ("b2", KC), ("g1", KC), ("be1", KC),
                    ("g2", KC), ("be2", KC)):
        din(name, [128, w], F32)
    din("Wwt", [128, 3 * KC], BF16)
    din("bwc", [1, 3], F32)
    din("epsc", [1, 1], F32)
    io["xs"] = nc.dram_tensor("xs", [DIM, tpc], BF16).ap()
    io["out"] = nc.dram_tensor("out", [tpc, 3], F32,
                               kind="ExternalOutput").ap()

    with tile.TileContext(nc) as tc:
        _emit(nc, tc, io, tpc)
    nc.compile()
    return nc


def _chunk_cols(vec, width):
    """[width*128] host vector -> [128, width] chunk-column layout."""
    return np.ascontiguousarray(vec.reshape(width, 128).T).astype(np.float32)


def prep_host_inputs(inputs, tpc=TPC, ncores=NCORES):
    """Build per-core input maps (host-side shard + transpose + bf16 cast)."""
    bf = ml_dtypes.bfloat16
    f32 = np.float32

    def fm(x):  # [B, DIM] -> [DIM, B] bf16 feature-major
        return np.ascontiguousarray(np.asarray(x, f32).T.astype(bf))

    m0 = fm(inputs["m0"]); m1 = fm(inputs["m1"]); m2 = fm(inputs["m2"])
    dom = fm(inputs["domain_rep"])

    # head-selector S[p, c*16+h] and expander E[h, c*128+p]
    head_of = np.arange(DIM) // HD
    S = np.zeros((128, 128), f32)
    E = np.zeros((16, 1024), f32)
    for c in range(KC):
        for p in range(128):
            h = head_of[c * 128 + p]
            S[p, c * 16 + h] = 1.0
            E[h, c * 128 + p] = 1.0

    consts = {
        "Wg": (np.asarray(inputs["Wg"], f32) / 3.0).astype(bf),
        "Wq": (np.asarray(inputs["Wq"], f32) / np.sqrt(HD)).astype(bf),
        "Wk": np.asarray(inputs["Wk"], f32).astype(bf),
        "Wv": np.asarray(inputs["Wv"], f32).astype(bf),
        "W1": np.asarray(inputs["W1"], f32).astype(bf),
        "W2": np.asarray(inputs["W2"], f32).astype(bf),
        "Ssel": S.astype(bf),
        "Eexp": E.astype(bf),
        "onecb": np.ones((128, 1), f32).astype(bf),
        "onerb": np.ones((1, 128), f32).astype(bf),
        "bg": _chunk_cols(np.asarray(inputs["bg"], f32), KC),
        "bq": _chunk_cols(np.asarray(inputs["bq"], f32) / np.sqrt(HD), KC),
        "bk": _chunk_cols(np.asarray(inputs["bk"], f32), KC),
        "bv": _chunk_cols(np.asarray(inputs["bv"], f32), KC),
        "b1": _chunk_cols(np.asarray(inputs["b1"], f32), MC1),
        "b2": _chunk_cols(np.asarray(inputs["b2"], f32), KC),
        "g1": _chunk_cols(np.asarray(inputs["g1"], f32), KC),
        "be1": _chunk_cols(np.asarray(inputs["beta1"], f32), KC),
        "g2": _chunk_cols(np.asarray(inputs["g2"], f32), KC),
        "be2": _chunk_cols(np.asarray(inputs["beta2"], f32), KC),
        "Wwt": np.ascontiguousarray(
            np.asarray(inputs["Ww"], f32).reshape(KC, 128, 3)
            .transpose(1, 0, 2).reshape(128, 3 * KC)).astype(bf),
        "bwc": np.asarray(inputs["bw"], f32).reshape(1, 3),
        "epsc": np.full((1, 1), EPS, f32),
    }

    in_maps = []
    for c in range(ncores):
        sl = slice(c * tpc, (c + 1) * tpc)
        m = dict(consts)
        m["m0"] = np.ascontiguousarray(m0[:, sl])
        m["m1"] = np.ascontiguousarray(m1[:, sl])
        m["m2"] = np.ascontiguousarray(m2[:, sl])
        m["dom"] = np.ascontiguousarray(dom[:, sl])
        in_maps.append(m)
    return in_maps


def kernel(**inputs):
    from concourse.bass_utils import run_bass_kernel_spmd
    nc = build_program()
    in_maps = prep_host_inputs(inputs)
    res = run_bass_kernel_spmd(nc, in_maps, list(range(NCORES)))
    out = np.concatenate([res.results[c]["out"] for c in range(NCORES)],
                         axis=0)
    return np.ascontiguousarray(out.astype(np.float32))



# revision 9
# speedup vs baseline: 1.0413x; 1.0413x over previous
"""Trainium2 Bass kernel for the fused 3-modality attention + FFN + softmax model.

Layout: pure data parallel over 8 NeuronCores (batch sharded); activations
feature-major ([1024 feats = 8 chunks x 128 partitions, tokens free]). GEMMs
in bf16 accumulating fp32 in PSUM.

v2 vs baseline:
- q-path fusion: q = dom@Wq' + (m0+m1+m2)@Wgq' with Wgq = Wg@Wq/3 precomputed
  host-side (removes the serial Wg->Wq PE/DVE round trip).
- scores accumulate across feature chunks directly in PSUM (no DVE adds).
- softmaxes skip max-subtraction (logits provably tiny).
- LN2 apply eliminated: logits computed from raw stats via
  z_c = rs*(x2@W'_c - mu*colsum(W'_c)) + b'_c with W' = g2*Ww folded host-side.
  Logit chains col-packed into one PSUM bank via tile_position.
- double-buffered per-block input tiles; per-mc weight tiles with host-side
  pre-transposed layout (contiguous 2KB DMA lines); weights on the gpsimd DMA
  queue, activations on sync, stores on scalar (parallel queues).
- warmup matmuls at t=0 keep the PE HAM clock-gate warm through the initial
  input DMA.
- output written as [3, tpc] (cheap row DMA), transposed on host.
"""

import numpy as np
import ml_dtypes

import concourse.bacc as bacc
import concourse.bass as bass
import concourse.mybir as mybir
import concourse.tile as tile

B, DIM, H, FFN, HD = 16384, 1024, 16, 4096, 64
NCORES = 8
TPC = B // NCORES          # tokens per core
TB = 512                   # token block (matmul moving dim)
KC = DIM // 128            # 8 feature chunks
MC1 = FFN // 128           # 32 ffn chunks
NBLK = TPC // TB
EPS = 1e-5
NWARM = 16                 # warmup matmuls at kernel start

BF16 = mybir.dt.bfloat16
F32 = mybir.dt.float32
AF = mybir.ActivationFunctionType
ALU = mybir.AluOpType


def _ln_apply(nc, pp, wk, src, out_bf, g, be, C):
    """LayerNorm over features; src (bf16) modified in place (phase A LN1)."""
    v, s, te = nc.vector, nc.scalar, nc.tensor
    pr1 = pp.tile([16, TB], F32, tag="sc3", bufs=3, name="pr1")
    for kc in range(KC):
        te.matmul(pr1[0:1, :], C["onecb"][:],
                  src[:, kc * TB:(kc + 1) * TB],
                  start=(kc == 0), stop=(kc == KC - 1))
    sq = wk.tile([128, KC * TB], BF16, tag="qb", bufs=1, name="sq")
    s.activation(sq[:], src[:], AF.Square)
    pr2 = pp.tile([16, TB], F32, tag="sc3", bufs=3, name="pr2")
    for kc in range(KC):
        te.matmul(pr2[0:1, :], C["onecb"][:], sq[:, kc * TB:(kc + 1) * TB],
                  start=(kc == 0), stop=(kc == KC - 1))
    mub = wk.tile([1, TB], BF16, tag="ln_mub", bufs=1, name="mub")[:]
    ex2 = wk.tile([1, TB], F32, tag="ln_s", bufs=3, name="ex2")[:]
    mu2 = wk.tile([1, TB], F32, tag="ln_s", bufs=3, name="mu2")[:]
    var = wk.tile([1, TB], F32, tag="ln_s", bufs=3, name="var")[:]
    sd = wk.tile([1, TB], F32, tag="ln_s", bufs=3, name="sd")[:]
    rs = wk.tile([1, TB], F32, tag="ln_s", bufs=3, name="rs")[:]
    rsb = wk.tile([1, TB], BF16, tag="ln_rsb", bufs=1, name="rsb")[:]
    s.activation(mub, pr1[0:1, :], AF.Copy, scale=1.0 / DIM)
    s.activation(ex2, pr2[0:1, :], AF.Copy, scale=1.0 / DIM)
    s.activation(mu2, mub, AF.Square)
    v.tensor_sub(var, ex2, mu2)
    s.activation(sd, var, AF.Sqrt, bias=C["epsc"][:])
    v.reciprocal(rs, sd)
    s.activation(rsb, rs, AF.Copy)
    pmu = pp.tile([128, TB], F32, tag="acc", bufs=4, name="pmu")
    te.matmul(pmu[:], C["onerb"][:], mub, start=True, stop=True)
    prs = pp.tile([128, TB], F32, tag="acc", bufs=4, name="prs")
    te.matmul(prs[:], C["onerb"][:], rsb, start=True, stop=True)
    for kc in range(KC):
        sl = src[:, kc * TB:(kc + 1) * TB]
        v.tensor_sub(sl, sl, pmu[:])
        v.tensor_mul(sl, sl, prs[:])
        s.activation(out_bf[:, kc * TB:(kc + 1) * TB], sl, AF.Identity,
                     scale=g[:, kc:kc + 1], bias=be[:, kc:kc + 1])


def _emit(nc, tc, io, tpc):
    nblk = tpc // TB
    v, s, te, g = nc.vector, nc.scalar, nc.tensor, nc.gpsimd

    with (
        tc.tile_pool(name="consts", bufs=1) as cp,
        tc.tile_pool(name="hold", bufs=1) as hp,
        tc.tile_pool(name="psum", bufs=1, space="PSUM") as pp,
    ):
        # ---- constants ----
        # Ssel first on the gpsimd queue (feeds warmup matmuls).
        C = {}
        t = cp.tile([128, 128], BF16, name="c_Ssel")
        g.dma_start(out=t[:], in_=io["Ssel"])
        C["Ssel"] = t
        # zer first on the sync queue (warmup rhs + et zero-fill).
        t = cp.tile([128, TB], BF16, name="c_zer")
        nc.sync.dma_start(out=t[:], in_=io["zer"])
        C["zer"] = t
        for name, shape, dtype in (
            ("Eexp", [16, 1024], BF16),
            ("onecb", [128, 1], BF16), ("onerb", [1, 128], BF16),
            ("bq2", [128, KC], F32),
            ("bk", [128, KC], F32), ("bv", [128, KC], F32),
            ("b1", [128, MC1], F32), ("b2", [128, KC], F32),
            ("g1", [128, KC], F32), ("be1", [128, KC], F32),
            ("Wwt", [128, 3 * KC], BF16), ("ucol", [128, 1], F32),
            ("bpcol", [128, 1], F32), ("sel3", [128, 1], BF16),
            ("epsc", [1, 1], F32),
        ):
            t = cp.tile(shape, dtype, name=f"c_{name}")
            nc.sync.dma_start(out=t[:], in_=io[name])
            C[name] = t

        # ---- warmup matmuls: keep the PE busy (HAM warm) during input DMA
        pw = pp.tile([128, TB], F32, tag="red", bufs=1, name="pw")
        for i in range(NWARM):
            te.matmul(pw[:], C["Ssel"][:], C["zer"][:], start=True, stop=True)

        # xhold: last phase-A block's LN1 output stays on chip
        xhold = hp.tile([128, KC * TB], BF16, name="xhold")

        # ---------------- phase A: attention + LN1 ----------------
        with (
            tc.tile_pool(name="wA", bufs=1) as wa,
            tc.tile_pool(name="workA", bufs=1) as wk,
        ):
            wmap = {}
            for wn in ("Wq", "Wgq", "Wk", "Wv"):
                tiles = []
                for mc in range(KC):
                    wt = wa.tile([128, KC * 128], BF16, name=f"w_{wn}_{mc}")
                    g.dma_start(out=wt[:],
                                in_=io[wn + "r"][mc * 128:(mc + 1) * 128, :])
                    tiles.append(wt)
                wmap[wn] = tiles

            def wsl(wn, kc, mc):
                return wmap[wn][mc][:, kc * 128:(kc + 1) * 128]

            for blk in range(nblk):
                t0 = blk * TB
                ins = {}
                for name in ("m0", "m1", "m2", "dom"):
                    t = wk.tile([128, KC * TB], BF16, tag=f"in_{name}",
                                bufs=2, name=f"{name}_sb")
                    nc.sync.dma_start(
                        out=t[:].rearrange("p (c t) -> p c t", t=TB),
                        in_=io[name].rearrange("(c p) t -> p c t",
                                               p=128)[:, :, t0:t0 + TB])
                    ins[name] = t
                mj = [ins["m0"], ins["m1"], ins["m2"]]
                dom = ins["dom"]

                # avg = m0+m1+m2, per-chunk (fine-grained deps)
                avg = wk.tile([128, KC * TB], BF16, tag="avg", bufs=1,
                              name="avg")
                for kc in range(KC):
                    sl = slice(kc * TB, (kc + 1) * TB)
                    v.tensor_add(avg[:, sl], mj[0][:, sl], mj[1][:, sl])
                    v.tensor_add(avg[:, sl], avg[:, sl], mj[2][:, sl])

                # q = dom@Wq' + avg@Wgq'  (both pre-scaled by 1/sqrt(HD))
                qb = wk.tile([128, KC * TB], BF16, tag="qb", bufs=1, name="qb")
                for mc in range(KC):
                    pq = pp.tile([128, TB], F32, tag="acc", bufs=4, name="pq")
                    for kc in range(KC):
                        te.matmul(pq[:], wsl("Wq", kc, mc),
                                  dom[:, kc * TB:(kc + 1) * TB],
                                  start=(kc == 0), stop=False)
                    for kc in range(KC):
                        te.matmul(pq[:], wsl("Wgq", kc, mc),
                                  avg[:, kc * TB:(kc + 1) * TB],
                                  start=False, stop=(kc == KC - 1))
                    s.activation(qb[:, mc * TB:(mc + 1) * TB], pq[:],
                                 AF.Identity, bias=C["bq2"][:, mc:mc + 1])

                # scores: per (mc) 3 modality K-groups; head-sums accumulate
                # across mc directly in PSUM (sc3 banks held for the loop)
                pss = [pp.tile([16, TB], F32, tag="sc3", bufs=3, name=f"ps{j}")
                       for j in range(3)]
                for mc in range(KC):
                    pks = [pp.tile([128, TB], F32, tag="acc", bufs=4,
                                   name=f"pk{j}") for j in range(3)]
                    for kc in range(KC):
                        for j in range(3):
                            te.matmul(pks[j][:], wsl("Wk", kc, mc),
                                      mj[j][:, kc * TB:(kc + 1) * TB],
                                      start=(kc == 0), stop=(kc == KC - 1))
                    for j in range(3):
                        tm = wk.tile([128, TB], BF16, tag="tm", bufs=2,
                                     name="tm")
                        v.scalar_tensor_tensor(
                            tm[:], pks[j][:], C["bk"][:, mc:mc + 1],
                            qb[:, mc * TB:(mc + 1) * TB],
                            ALU.add, ALU.mult)
                        te.matmul(pss[j][0:16, :],
                                  C["Ssel"][:, mc * 16:(mc + 1) * 16],
                                  tm[:], start=(mc == 0), stop=(mc == KC - 1))

                # softmax over 3 modalities (no max-sub; |score| < ~6)
                ab = wk.tile([16, 3 * TB], BF16, tag="ab", bufs=1, name="ab")
                for j in range(3):
                    s.activation(ab[:, j * TB:(j + 1) * TB], pss[j][0:16, :],
                                 AF.Exp)
                sm = wk.tile([16, TB], F32, tag="sm", bufs=1, name="sm")[:]
                rc = wk.tile([16, TB], F32, tag="rc", bufs=1, name="rc")[:]
                v.tensor_add(sm, ab[:, 0:TB], ab[:, TB:2 * TB])
                v.tensor_add(sm, sm, ab[:, 2 * TB:3 * TB])
                v.reciprocal(rc, sm)
                for j in range(3):
                    abj = ab[:, j * TB:(j + 1) * TB]
                    v.tensor_mul(abj, abj, rc)

                # attnout = sum_j bcast(attn_j) * (m_j @ Wv); +bv+dom fused
                xp = wk.tile([128, KC * TB], BF16, tag="xp", bufs=1, name="xp")
                for mc in range(KC):
                    pvs = [pp.tile([128, TB], F32, tag="acc", bufs=4,
                                   name=f"pv{j}") for j in range(3)]
                    for kc in range(KC):
                        for j in range(3):
                            te.matmul(pvs[j][:], wsl("Wv", kc, mc),
                                      mj[j][:, kc * TB:(kc + 1) * TB],
                                      start=(kc == 0), stop=(kc == KC - 1))
                    acc = wk.tile([128, TB], F32, tag="acc_s", bufs=2,
                                  name="acc")
                    for j in range(3):
                        vt = wk.tile([128, TB], BF16, tag="vt", bufs=2,
                                     name="vt")
                        s.activation(vt[:], pvs[j][:], AF.Copy)
                        pa = pp.tile([128, TB], F32, tag="sc3", bufs=3,
                                     name="pa")
                        te.matmul(pa[:], C["Eexp"][:, mc * 128:(mc + 1) * 128],
                                  ab[:, j * TB:(j + 1) * TB],
                                  start=True, stop=True)
                        if j == 0:
                            v.tensor_mul(acc[:], pa[:], vt[:])
                        else:
                            t2 = wk.tile([128, TB], F32, tag="t2", bufs=2,
                                         name="t2")
                            v.tensor_mul(t2[:], pa[:], vt[:])
                            v.tensor_add(acc[:], acc[:], t2[:])
                    v.scalar_tensor_tensor(
                        xp[:, mc * TB:(mc + 1) * TB], acc[:],
                        C["bv"][:, mc:mc + 1], dom[:, mc * TB:(mc + 1) * TB],
                        ALU.add, ALU.add)

                if blk == nblk - 1:
                    _ln_apply(nc, pp, wk, xp, xhold[:], C["g1"], C["be1"], C)
                else:
                    x_bf = wk.tile([128, KC * TB], BF16, tag="xbf", bufs=1,
                                   name="x_bf")
                    _ln_apply(nc, pp, wk, xp, x_bf[:], C["g1"], C["be1"], C)
                    s.dma_start(
                        out=io["xs"].rearrange("(c p) t -> p c t",
                                               p=128)[:, :, t0:t0 + TB],
                        in_=x_bf[:].rearrange("p (c t) -> p c t", t=TB))

        # ---------------- phase B: FFN + logits from stats ----------------
        with (
            tc.tile_pool(name="wB", bufs=1) as wb,
            tc.tile_pool(name="workB", bufs=1) as wk,
        ):
            w1t = []
            for mc in range(MC1):
                t = wb.tile([128, KC * 128], BF16, name=f"w_W1_{mc}")
                g.dma_start(out=t[:],
                            in_=io["W1r"][mc * 128:(mc + 1) * 128, :])
                w1t.append(t)

            # et: class exponentials live at partitions 0/32/64; other lanes
            # must be zero for the selector gather matmul. Zero once.
            et = wk.tile([65, TB], BF16, tag="et", bufs=1, name="et")
            v.tensor_copy(et[:], C["zer"][0:65, :])

            for blk in range(nblk):
                t0 = blk * TB
                if blk == nblk - 1:
                    xb = xhold
                else:
                    xb = wk.tile([128, KC * TB], BF16, tag="xb", bufs=2,
                                 name="xb")
                    nc.sync.dma_start(
                        out=xb[:].rearrange("p (c t) -> p c t", t=TB),
                        in_=io["xs"].rearrange("(c p) t -> p c t",
                                               p=128)[:, :, t0:t0 + TB])
                hb = wk.tile([128, MC1 * TB], BF16, tag="hb", bufs=2,
                             name="hb")
                for mc in range(MC1):
                    ph = pp.tile([128, TB], F32, tag="acc", bufs=4, name="ph")
                    for kc in range(KC):
                        te.matmul(ph[:],
                                  w1t[mc][:, kc * 128:(kc + 1) * 128],
                                  xb[:, kc * TB:(kc + 1) * TB],
                                  start=(kc == 0), stop=(kc == KC - 1))
                    s.activation(hb[:, mc * TB:(mc + 1) * TB], ph[:], AF.Relu,
                                 bias=C["b1"][:, mc:mc + 1])

                x2 = wk.tile([128, KC * TB], BF16, tag="x2", bufs=1, name="x2")
                for mc in range(KC):
                    w2t = wk.tile([128, MC1 * 128], BF16, tag="w2t", bufs=2,
                                  name="w2t")
                    g.dma_start(out=w2t[:],
                                in_=io["W2r"][mc * 128:(mc + 1) * 128, :])
                    pf = pp.tile([128, TB], F32, tag="acc", bufs=4, name="pf")
                    for kc in range(MC1):
                        te.matmul(pf[:], w2t[:, kc * 128:(kc + 1) * 128],
                                  hb[:, kc * TB:(kc + 1) * TB],
                                  start=(kc == 0), stop=(kc == MC1 - 1))
                    v.scalar_tensor_tensor(
                        x2[:, mc * TB:(mc + 1) * TB], pf[:],
                        C["b2"][:, mc:mc + 1], xb[:, mc * TB:(mc + 1) * TB],
                        ALU.add, ALU.add)

                # stats: sums + col-packed logit chains A_c = x2 @ W'_c
                sq = wk.tile([128, KC * TB], BF16, tag="sq", bufs=1, name="sq")
                s.activation(sq[:], x2[:], AF.Square)
                pr1 = pp.tile([16, TB], F32, tag="sc3", bufs=3, name="pr1")
                for kc in range(KC):
                    te.matmul(pr1[0:1, :], C["onecb"][:],
                              x2[:, kc * TB:(kc + 1) * TB],
                              start=(kc == 0), stop=(kc == KC - 1))
                pr2 = pp.tile([16, TB], F32, tag="sc3", bufs=3, name="pr2")
                for kc in range(KC):
                    te.matmul(pr2[0:1, :], C["onecb"][:],
                              sq[:, kc * TB:(kc + 1) * TB],
                              start=(kc == 0), stop=(kc == KC - 1))
                psA = pp.tile([128, TB], F32, tag="red", bufs=1, name="psA")
                for c in range(3):
                    for kc in range(KC):
                        te.matmul(psA[32 * c:32 * c + 1, :],
                                  C["Wwt"][:, c * KC + kc:c * KC + kc + 1],
                                  x2[:, kc * TB:(kc + 1) * TB],
                                  tile_position=(0, 32 * c),
                                  start=(kc == 0), stop=(kc == KC - 1))

                # scalar chain -> mu, -rs (bf16 rows at base 0)
                mub = wk.tile([1, TB], BF16, tag="f_mub", bufs=1, name="mub")[:]
                ex2 = wk.tile([1, TB], F32, tag="f_s", bufs=3, name="ex2")[:]
                mu2 = wk.tile([1, TB], F32, tag="f_s", bufs=3, name="mu2")[:]
                var = wk.tile([1, TB], F32, tag="f_s", bufs=3, name="var")[:]
                sd = wk.tile([1, TB], F32, tag="f_s", bufs=3, name="sd")[:]
                rs = wk.tile([1, TB], F32, tag="f_s", bufs=3, name="rs")[:]
                nrb = wk.tile([1, TB], BF16, tag="f_nrb", bufs=1, name="nrb")[:]
                s.activation(mub, pr1[0:1, :], AF.Copy, scale=1.0 / DIM)
                s.activation(ex2, pr2[0:1, :], AF.Copy, scale=1.0 / DIM)
                s.activation(mu2, mub, AF.Square)
                v.tensor_sub(var, ex2, mu2)
                s.activation(sd, var, AF.Sqrt, bias=C["epsc"][:])
                v.reciprocal(rs, sd)
                s.activation(nrb, rs, AF.Copy, scale=-1.0)

                # broadcast mu and -rs to all 128 partitions (K=1 expands)
                pmu = pp.tile([128, TB], F32, tag="acc", bufs=4, name="pmu")
                te.matmul(pmu[:], C["onerb"][:], mub, start=True, stop=True)
                pnr = pp.tile([128, TB], F32, tag="acc", bufs=4, name="pnr")
                te.matmul(pnr[:], C["onerb"][:], nrb, start=True, stop=True)
                smu = wk.tile([65, TB], BF16, tag="smu", bufs=1, name="smu")
                s.activation(smu[:], pmu[0:65, :], AF.Copy)
                snr = wk.tile([65, TB], BF16, tag="snr", bufs=1, name="snr")
                s.activation(snr[:], pnr[0:65, :], AF.Copy)

                # z_c = rs*(A_c - mu*u_c) + b'_c ; e_c = exp(z_c)
                zt = wk.tile([65, TB], F32, tag="zt", bufs=1, name="zt")
                for c in range(3):
                    r = slice(32 * c, 32 * c + 1)
                    v.scalar_tensor_tensor(zt[r, :], smu[r, :],
                                           C["ucol"][r, :], psA[r, :],
                                           ALU.mult, ALU.subtract)
                    v.tensor_mul(zt[r, :], zt[r, :], snr[r, :])
                    s.activation(et[r, :], zt[r, :], AF.Exp,
                                 bias=C["bpcol"][r, :])

                # p_c = e_c / sum_c e_c via selector-gather + K=1 expand
                gsum = pp.tile([16, TB], F32, tag="sc3", bufs=3, name="gsum")
                te.matmul(gsum[0:1, :], C["sel3"][0:65, :], et[:],
                          start=True, stop=True)
                rr = wk.tile([1, TB], F32, tag="f_rr", bufs=1, name="rr")[:]
                rrb = wk.tile([1, TB], BF16, tag="f_rrb", bufs=1, name="rrb")[:]
                v.reciprocal(rr, gsum[0:1, :])
                s.activation(rrb, rr, AF.Copy)
                prr = pp.tile([128, TB], F32, tag="acc", bufs=4, name="prr")
                te.matmul(prr[:], C["onerb"][:], rrb, start=True, stop=True)
                pot = wk.tile([65, TB], F32, tag="zt", bufs=1, name="pot")
                for c in range(3):
                    r = slice(32 * c, 32 * c + 1)
                    v.tensor_mul(pot[r, :], et[r, :], prr[r, :])
                    s.dma_start(out=io["outT"][c:c + 1, t0:t0 + TB],
                                in_=pot[r, :])


def build_program(tpc=TPC):
    nc = bacc.Bacc("TRN2", target_bir_lowering=False, debug=False)
    io = {}

    def din(name, shape, dtype, kind="ExternalInput"):
        io[name] = nc.dram_tensor(name, shape, dtype, kind=kind).ap()

    for name in ("m0", "m1", "m2", "dom"):
        din(name, [DIM, tpc], BF16)
    for name in ("Wqr", "Wgqr", "Wkr", "Wvr"):
        din(name, [DIM, DIM], BF16)
    din("W1r", [FFN, DIM], BF16)
    din("W2r", [DIM, FFN], BF16)
    din("Ssel", [128, 128], BF16)
    din("zer", [128, TB], BF16)
    din("Eexp", [16, 1024], BF16)
    din("onecb", [128, 1], BF16)
    din("onerb", [1, 128], BF16)
    for name, w in (("bq2", KC), ("bk", KC), ("bv", KC),
                    ("b1", MC1), ("b2", KC), ("g1", KC), ("be1", KC)):
        din(name, [128, w], F32)
    din("Wwt", [128, 3 * KC], BF16)
    din("ucol", [128, 1], F32)
    din("bpcol", [128, 1], F32)
    din("sel3", [128, 1], BF16)
    din("epsc", [1, 1], F32)
    io["xs"] = nc.dram_tensor("xs", [DIM, tpc], BF16).ap()
    io["outT"] = nc.dram_tensor("outT", [3, tpc], F32,
                                kind="ExternalOutput").ap()

    with tile.TileContext(nc) as tc:
        _emit(nc, tc, io, tpc)
    nc.compile()
    return nc


def _chunk_cols(vec, width):
    """[width*128] host vector -> [128, width] chunk-column layout."""
    return np.ascontiguousarray(vec.reshape(width, 128).T).astype(np.float32)


def _wr(Wf32, nkc, nmc):
    """[nkc*128, nmc*128] -> [nmc*128, nkc*128] per-mc stationary layout:
    out[mc*128+p, kc*128+j] = W[kc*128+p, mc*128+j]."""
    bf = ml_dtypes.bfloat16
    W = Wf32.reshape(nkc, 128, nmc, 128).transpose(2, 1, 0, 3)
    return np.ascontiguousarray(W.reshape(nmc * 128, nkc * 128)).astype(bf)


def prep_host_inputs(inputs, tpc=TPC, ncores=NCORES):
    """Per-core input maps (host-side shard + transpose + bf16 + folds)."""
    bf = ml_dtypes.bfloat16
    f32 = np.float32

    def fm(x):  # [B, DIM] -> [DIM, B] bf16 feature-major
        return np.ascontiguousarray(np.asarray(x, f32).T.astype(bf))

    m0 = fm(inputs["m0"]); m1 = fm(inputs["m1"]); m2 = fm(inputs["m2"])
    dom = fm(inputs["domain_rep"])

    head_of = np.arange(DIM) // HD
    S = np.zeros((128, 128), f32)
    E = np.zeros((16, 1024), f32)
    for c in range(KC):
        for p in range(128):
            h = head_of[c * 128 + p]
            S[p, c * 16 + h] = 1.0
            E[h, c * 128 + p] = 1.0

    Wg = np.asarray(inputs["Wg"], f32); Wq = np.asarray(inputs["Wq"], f32)
    sc = 1.0 / np.sqrt(HD)
    Wq_s = Wq * sc
    Wgq_s = (Wg @ Wq) * (sc / 3.0)
    bq2 = (np.asarray(inputs["bg"], f32) @ Wq
           + np.asarray(inputs["bq"], f32)) * sc

    # logits fold: W' = g2*Ww, u_c = colsum(bf16(W')), b'_c = beta2@Ww + bw
    g2 = np.asarray(inputs["g2"], f32)
    Ww = np.asarray(inputs["Ww"], f32)
    Wp = (g2[:, None] * Ww)
    Wp_bf = Wp.astype(bf)
    u = Wp_bf.astype(f32).sum(axis=0)                      # [3]
    bp = np.asarray(inputs["beta2"], f32) @ Ww + np.asarray(inputs["bw"], f32)
    # Wwt[p, c*KC+kc] = W'[kc*128+p, c]
    Wwt = np.ascontiguousarray(
        Wp.reshape(KC, 128, 3).transpose(1, 2, 0).reshape(128, 3 * KC)
    ).astype(bf)
    ucol = np.zeros((128, 1), f32)
    bpcol = np.zeros((128, 1), f32)
    sel3 = np.zeros((128, 1), f32)
    for c in range(3):
        ucol[32 * c, 0] = u[c]
        bpcol[32 * c, 0] = bp[c]
        sel3[32 * c, 0] = 1.0

    consts = {
        "Wqr": _wr(Wq_s, KC, KC),
        "Wgqr": _wr(Wgq_s, KC, KC),
        "Wkr": _wr(np.asarray(inputs["Wk"], f32), KC, KC),
        "Wvr": _wr(np.asarray(inputs["Wv"], f32), KC, KC),
        "W1r": _wr(np.asarray(inputs["W1"], f32), KC, MC1),
        "W2r": _wr(np.asarray(inputs["W2"], f32), MC1, KC),
        "Ssel": S.astype(bf),
        "zer": np.zeros((128, TB), f32).astype(bf),
        "Eexp": E.astype(bf),
        "onecb": np.ones((128, 1), f32).astype(bf),
        "onerb": np.ones((1, 128), f32).astype(bf),
        "bq2": _chunk_cols(bq2, KC),
        "bk": _chunk_cols(np.asarray(inputs["bk"], f32), KC),
        "bv": _chunk_cols(np.asarray(inputs["bv"], f32), KC),
        "b1": _chunk_cols(np.asarray(inputs["b1"], f32), MC1),
        "b2": _chunk_cols(np.asarray(inputs["b2"], f32), KC),
        "g1": _chunk_cols(np.asarray(inputs["g1"], f32), KC),
        "be1": _chunk_cols(np.asarray(inputs["beta1"], f32), KC),
        "Wwt": Wwt,
        "ucol": ucol,
        "bpcol": bpcol,
        "sel3": sel3.astype(bf),
        "epsc": np.full((1, 1), EPS, f32),
    }

    in_maps = []
    for c in range(ncores):
        sl = slice(c * tpc, (c + 1) * tpc)
        m = dict(consts)
        m["m0"] = np.ascontiguousarray(m0[:, sl])
        m["m1"] = np.ascontiguousarray(m1[:, sl])
        m["m2"] = np.ascontiguousarray(m2[:, sl])
        m["dom"] = np.ascontiguousarray(dom[:, sl])
        in_maps.append(m)
    return in_maps


def kernel(**inputs):
    from concourse.bass_utils import run_bass_kernel_spmd
    nc = build_program()
    in_maps = prep_host_inputs(inputs)
    res = run_bass_kernel_spmd(nc, in_maps, list(range(NCORES)))
    out = np.concatenate(
        [res.results[c]["outT"].T for c in range(NCORES)], axis=0)
    return np.ascontiguousarray(out.astype(np.float32))


# revision 14
# speedup vs baseline: 1.1385x; 1.0933x over previous
"""Trainium2 Bass kernel for the fused 3-modality attention + FFN + softmax model.

Layout: pure data parallel over 8 NeuronCores (batch sharded); activations
feature-major ([1024 feats = 8 chunks x 128 partitions, tokens free]). GEMMs
in bf16 accumulating fp32 in PSUM.

v3 structure:
- q-path fusion: q = dom@Wq' + (m0+m1+m2)@Wgq' with Wgq = Wg@Wq/3 folded
  host-side (no serial Wg->Wq round trip).
- software-pipelined emission: each block's LayerNorm/stats tails are emitted
  AFTER the next block's big matmul groups, so the in-order PE stream never
  waits on the serial scalar/vector stat chains.
- all cross-partition broadcasts (LN mu/rs, logit scalars) on GPSIMD
  partition_broadcast; final 3-way softmax normalization via GPSIMD
  partition_all_reduce. Zero PE involvement in those chains.
- LN2 apply eliminated: logits from raw stats, z_c = rs*(A_c - mu*u_c) + b'_c
  with W' = g2*Ww folded host-side; A_c chains col-packed in one PSUM bank.
- softmaxes skip max-subtraction (logits provably tiny).
- scores accumulate across feature chunks directly in PSUM.
- weights on the gpsimd DMA queue (interleaved per-mc order), activations on
  sync, stores on scalar; warmup matmuls cover the initial DMA window.
- output written as [3, tpc] rows, transposed on host.
"""

import numpy as np
import ml_dtypes

import concourse.bacc as bacc
import concourse.bass as bass
import concourse.bass_isa as bass_isa
import concourse.mybir as mybir
import concourse.tile as tile

B, DIM, H, FFN, HD = 16384, 1024, 16, 4096, 64
NCORES = 8
TPC = B // NCORES          # tokens per core
TB = 512                   # token block (matmul moving dim)
KC = DIM // 128            # 8 feature chunks
MC1 = FFN // 128           # 32 ffn chunks
NBLK = TPC // TB
EPS = 1e-5
NWARM = 16                 # warmup matmuls at kernel start

BF16 = mybir.dt.bfloat16
F32 = mybir.dt.float32
AF = mybir.ActivationFunctionType
ALU = mybir.AluOpType


def _emit(nc, tc, io, tpc):
    nblk = tpc // TB
    v, s, te, gp = nc.vector, nc.scalar, nc.tensor, nc.gpsimd

    with (
        tc.tile_pool(name="consts", bufs=1) as cp,
        tc.tile_pool(name="hold", bufs=1) as hp,
        tc.tile_pool(name="psum", bufs=1, space="PSUM") as pp,
    ):
        # ---- constants ----
        C = {}
        t = cp.tile([128, 128], BF16, name="c_Ssel")
        gp.dma_start(out=t[:], in_=io["Ssel"])          # first on gpsimd q
        C["Ssel"] = t
        t = cp.tile([128, TB], BF16, name="c_zer")
        nc.sync.dma_start(out=t[:], in_=io["zer"])      # first on sync q
        C["zer"] = t
        for name, shape, dtype in (
            ("Eexp", [16, 1024], BF16), ("onecb", [128, 1], BF16),
            ("bq2", [128, KC], F32),
            ("bk", [128, KC], F32), ("bv", [128, KC], F32),
            ("b1", [128, MC1], F32), ("b2", [128, KC], F32),
            ("g1", [128, KC], F32), ("be1", [128, KC], F32),
            ("Wwt", [128, 3 * KC], BF16), ("ucol", [128, 1], F32),
            ("bpcol", [128, 1], F32), ("epsc", [1, 1], F32),
        ):
            t = cp.tile(shape, dtype, name=f"c_{name}")
            nc.sync.dma_start(out=t[:], in_=io[name])
            C[name] = t

        # ---- warmup: keep the PE HAM clock-gate warm through input DMA
        pw = pp.tile([128, TB], F32, tag="red", bufs=1, name="pw")
        for i in range(NWARM):
            te.matmul(pw[:], C["Ssel"][:], C["zer"][:], start=True, stop=True)

        # xhold: last phase-A block's LN1 output stays on chip for phase B
        xhold = hp.tile([128, KC * TB], BF16, name="xhold")

        # ---------------- phase A: attention + LN1 ----------------
        wk_cm = tc.tile_pool(name="workA", bufs=1)
        wk = wk_cm.__enter__()
        wa_cm = tc.tile_pool(name="wA", bufs=1)
        wa = wa_cm.__enter__()

        wmap = {wn: [None] * KC for wn in ("Wq", "Wgq", "Wk", "Wv")}

        def wload(wn, mc):
            wt = wa.tile([128, KC * 128], BF16, name=f"w_{wn}_{mc}")
            gp.dma_start(out=wt[:],
                         in_=io[wn + "r"][mc * 128:(mc + 1) * 128, :])
            wmap[wn][mc] = wt

        for mc in range(KC):            # q/k weights interleaved per-mc
            for wn in ("Wq", "Wgq", "Wk"):
                wload(wn, mc)
        for mc in range(KC):
            wload("Wv", mc)

        def wsl(wn, kc, mc):
            return wmap[wn][mc][:, kc * 128:(kc + 1) * 128]

        def emit_inputs(blk):
            t0 = blk * TB
            ins = {}
            for name in ("m0", "m1", "m2", "dom"):
                t = wk.tile([128, KC * TB], BF16, tag=f"in_{name}",
                            bufs=2, name=f"{name}_sb")
                nc.sync.dma_start(
                    out=t[:].rearrange("p (c t) -> p c t", t=TB),
                    in_=io[name].rearrange("(c p) t -> p c t",
                                           p=128)[:, :, t0:t0 + TB])
                ins[name] = t
            avg = wk.tile([128, KC * TB], BF16, tag="avg", bufs=1, name="avg")
            for kc in range(KC):
                sl = slice(kc * TB, (kc + 1) * TB)
                v.tensor_add(avg[:, sl], ins["m0"][:, sl], ins["m1"][:, sl])
                v.tensor_add(avg[:, sl], avg[:, sl], ins["m2"][:, sl])
            ins["avg"] = avg
            return ins

        def emit_q(ins):
            qb = wk.tile([128, KC * TB], BF16, tag="qb", bufs=1, name="qb")
            for mc in range(KC):
                pq = pp.tile([128, TB], F32, tag="acc", bufs=4, name="pq")
                for kc in range(KC):
                    te.matmul(pq[:], wsl("Wgq", kc, mc),
                              ins["avg"][:, kc * TB:(kc + 1) * TB],
                              start=(kc == 0), stop=False)
                for kc in range(KC):
                    te.matmul(pq[:], wsl("Wq", kc, mc),
                              ins["dom"][:, kc * TB:(kc + 1) * TB],
                              start=False, stop=(kc == KC - 1))
                s.activation(qb[:, mc * TB:(mc + 1) * TB], pq[:],
                             AF.Identity, bias=C["bq2"][:, mc:mc + 1])
            return qb

        def emit_kv(ins, qb):
            mj = [ins["m0"], ins["m1"], ins["m2"]]
            dom = ins["dom"]
            # scores accumulate across mc in held PSUM banks
            pss = [pp.tile([16, TB], F32, tag="sc3", bufs=3, name=f"ps{j}")
                   for j in range(3)]
            for mc in range(KC):
                pks = [pp.tile([128, TB], F32, tag="acc", bufs=4,
                               name=f"pk{j}") for j in range(3)]
                for kc in range(KC):
                    for j in range(3):
                        te.matmul(pks[j][:], wsl("Wk", kc, mc),
                                  mj[j][:, kc * TB:(kc + 1) * TB],
                                  start=(kc == 0), stop=(kc == KC - 1))
                for j in range(3):
                    tm = wk.tile([128, TB], BF16, tag="tm", bufs=2, name="tm")
                    v.scalar_tensor_tensor(
                        tm[:], pks[j][:], C["bk"][:, mc:mc + 1],
                        qb[:, mc * TB:(mc + 1) * TB], ALU.add, ALU.mult)
                    te.matmul(pss[j][0:16, :],
                              C["Ssel"][:, mc * 16:(mc + 1) * 16],
                              tm[:], start=(mc == 0), stop=(mc == KC - 1))
            # softmax over 3 modalities (no max-sub; scores provably small)
            ab = wk.tile([16, 3 * TB], BF16, tag="ab", bufs=1, name="ab")
            for j in range(3):
                s.activation(ab[:, j * TB:(j + 1) * TB], pss[j][0:16, :],
                             AF.Exp)
            sm = wk.tile([16, TB], F32, tag="sm", bufs=1, name="sm")[:]
            rc = wk.tile([16, TB], F32, tag="rc", bufs=1, name="rc")[:]
            v.tensor_add(sm, ab[:, 0:TB], ab[:, TB:2 * TB])
            v.tensor_add(sm, sm, ab[:, 2 * TB:3 * TB])
            v.reciprocal(rc, sm)
            for j in range(3):
                abj = ab[:, j * TB:(j + 1) * TB]
                v.tensor_mul(abj, abj, rc)

            xp = wk.tile([128, KC * TB], BF16, tag="xp", bufs=1, name="xp")
            for mc in range(KC):
                pvs = [pp.tile([128, TB], F32, tag="acc", bufs=4,
                               name=f"pv{j}") for j in range(3)]
                for kc in range(KC):
                    for j in range(3):
                        te.matmul(pvs[j][:], wsl("Wv", kc, mc),
                                  mj[j][:, kc * TB:(kc + 1) * TB],
                                  start=(kc == 0), stop=(kc == KC - 1))
                acc = wk.tile([128, TB], F32, tag="acc_s", bufs=1, name="acc")
                for j in range(3):
                    vt = wk.tile([128, TB], BF16, tag="vt", bufs=2, name="vt")
                    s.activation(vt[:], pvs[j][:], AF.Copy)
                    pa = pp.tile([128, TB], F32, tag="sc3", bufs=3, name="pa")
                    te.matmul(pa[:], C["Eexp"][:, mc * 128:(mc + 1) * 128],
                              ab[:, j * TB:(j + 1) * TB],
                              start=True, stop=True)
                    if j == 0:
                        v.tensor_mul(acc[:], pa[:], vt[:])
                    else:
                        t2 = wk.tile([128, TB], F32, tag="t2", bufs=1,
                                     name="t2")
                        v.tensor_mul(t2[:], pa[:], vt[:])
                        v.tensor_add(acc[:], acc[:], t2[:])
                v.scalar_tensor_tensor(
                    xp[:, mc * TB:(mc + 1) * TB], acc[:],
                    C["bv"][:, mc:mc + 1], dom[:, mc * TB:(mc + 1) * TB],
                    ALU.add, ALU.add)
            return xp

        def emit_ln1(blk, xp):
            # stats matmuls (these sit after the NEXT block's q-group in the
            # PE stream, so their DVE/scalar inputs are long since ready)
            pr1 = pp.tile([16, TB], F32, tag="sc3", bufs=3, name="pr1")
            for kc in range(KC):
                te.matmul(pr1[0:1, :], C["onecb"][:],
                          xp[:, kc * TB:(kc + 1) * TB],
                          start=(kc == 0), stop=(kc == KC - 1))
            sq = wk.tile([128, KC * TB], BF16, tag="sqA", bufs=1, name="sq")
            s.activation(sq[:], xp[:], AF.Square)
            pr2 = pp.tile([16, TB], F32, tag="sc3", bufs=3, name="pr2")
            for kc in range(KC):
                te.matmul(pr2[0:1, :], C["onecb"][:],
                          sq[:, kc * TB:(kc + 1) * TB],
                          start=(kc == 0), stop=(kc == KC - 1))
            mub = wk.tile([1, TB], BF16, tag="ln_mub", bufs=1, name="mub")[:]
            mu2 = wk.tile([1, TB], F32, tag="ln_s", bufs=3, name="mu2")[:]
            var = wk.tile([1, TB], F32, tag="ln_s", bufs=3, name="var")[:]
            sd = wk.tile([1, TB], F32, tag="ln_s", bufs=3, name="sd")[:]
            rs = wk.tile([1, TB], F32, tag="ln_s", bufs=3, name="rs")[:]
            rsb = wk.tile([1, TB], BF16, tag="ln_rsb", bufs=1, name="rsb")[:]
            s.activation(mub, pr1[0:1, :], AF.Copy, scale=1.0 / DIM)
            s.activation(mu2, mub, AF.Square)
            v.scalar_tensor_tensor(var, pr2[0:1, :], 1.0 / DIM, mu2,
                                   ALU.mult, ALU.subtract)
            s.activation(sd, var, AF.Sqrt, bias=C["epsc"][:])
            v.reciprocal(rs, sd)
            s.activation(rsb, rs, AF.Copy)
            smu = wk.tile([128, TB], BF16, tag="smu", bufs=1, name="smu")
            gp.partition_broadcast(smu[:], mub, channels=128)
            srs = wk.tile([128, TB], BF16, tag="srs", bufs=1, name="srs")
            gp.partition_broadcast(srs[:], rsb, channels=128)
            out_bf = (xhold if blk == nblk - 1 else
                      wk.tile([128, KC * TB], BF16, tag="xbf", bufs=1,
                              name="x_bf"))
            for kc in range(KC):
                sl = xp[:, kc * TB:(kc + 1) * TB]
                v.tensor_sub(sl, sl, smu[:])
                v.tensor_mul(sl, sl, srs[:])
                s.activation(out_bf[:, kc * TB:(kc + 1) * TB], sl, AF.Identity,
                             scale=C["g1"][:, kc:kc + 1],
                             bias=C["be1"][:, kc:kc + 1])
            if blk != nblk - 1:
                t0 = blk * TB
                s.dma_start(
                    out=io["xs"].rearrange("(c p) t -> p c t",
                                           p=128)[:, :, t0:t0 + TB],
                    in_=out_bf[:].rearrange("p (c t) -> p c t", t=TB))

        # software pipeline: LN1(prev) lands after q(blk)
        ins_cur = emit_inputs(0)
        prev_xp = None
        prev_blk = -1
        for blk in range(nblk):
            qb = emit_q(ins_cur)
            if prev_xp is not None:
                emit_ln1(prev_blk, prev_xp)
            ins_nxt = emit_inputs(blk + 1) if blk + 1 < nblk else None
            prev_xp = emit_kv(ins_cur, qb)
            prev_blk = blk
            ins_cur = ins_nxt
        wa_cm.__exit__(None, None, None)      # free weight SBUF for phase B
        emit_ln1(prev_blk, prev_xp)
        wk_cm.__exit__(None, None, None)

        # ---------------- phase B: FFN + logits from stats ----------------
        with (
            tc.tile_pool(name="wB", bufs=1) as wb,
            tc.tile_pool(name="workB", bufs=1) as wk2,
        ):
            wk = wk2
            w1t = []
            for mc in range(MC1):
                t = wb.tile([128, KC * 128], BF16, name=f"w_W1_{mc}")
                gp.dma_start(out=t[:],
                             in_=io["W1r"][mc * 128:(mc + 1) * 128, :])
                w1t.append(t)

            # class exponentials at partitions 0/32/64; other lanes must stay
            # zero for the partition_all_reduce sum. Zero once.
            et = wk.tile([65, TB], BF16, tag="et", bufs=1, name="et")
            v.tensor_copy(et[:], C["zer"][0:65, :])

            def emit_xb(blk):
                if blk == nblk - 1:
                    return xhold
                t0 = blk * TB
                xb = wk.tile([128, KC * TB], BF16, tag="xb", bufs=2,
                             name="xb")
                nc.sync.dma_start(
                    out=xb[:].rearrange("p (c t) -> p c t", t=TB),
                    in_=io["xs"].rearrange("(c p) t -> p c t",
                                           p=128)[:, :, t0:t0 + TB])
                return xb

            def emit_w1(xb):
                hb = wk.tile([128, MC1 * TB], BF16, tag="hb", bufs=2,
                             name="hb")
                for mc in range(MC1):
                    ph = pp.tile([128, TB], F32, tag="acc", bufs=4, name="ph")
                    for kc in range(KC):
                        te.matmul(ph[:],
                                  w1t[mc][:, kc * 128:(kc + 1) * 128],
                                  xb[:, kc * TB:(kc + 1) * TB],
                                  start=(kc == 0), stop=(kc == KC - 1))
                    s.activation(hb[:, mc * TB:(mc + 1) * TB], ph[:], AF.Relu,
                                 bias=C["b1"][:, mc:mc + 1])
                return hb

            def emit_w2(xb, hb):
                x2 = wk.tile([128, KC * TB], BF16, tag="x2", bufs=1,
                             name="x2")
                sq = wk.tile([128, KC * TB], BF16, tag="sqB", bufs=1,
                             name="sqB")
                for mc in range(KC):
                    w2t = wk.tile([128, MC1 * 128], BF16, tag="w2t", bufs=2,
                                  name="w2t")
                    gp.dma_start(out=w2t[:],
                                 in_=io["W2r"][mc * 128:(mc + 1) * 128, :])
                    pf = pp.tile([128, TB], F32, tag="acc", bufs=4, name="pf")
                    for kc in range(MC1):
                        te.matmul(pf[:], w2t[:, kc * 128:(kc + 1) * 128],
                                  hb[:, kc * TB:(kc + 1) * TB],
                                  start=(kc == 0), stop=(kc == MC1 - 1))
                    msl = slice(mc * TB, (mc + 1) * TB)
                    v.scalar_tensor_tensor(
                        x2[:, msl], pf[:], C["b2"][:, mc:mc + 1], xb[:, msl],
                        ALU.add, ALU.add)
                    s.activation(sq[:, msl], x2[:, msl], AF.Square)
                return x2, sq

            def emit_stats1(x2, sq):
                """Stat + logit matmuls; placed after the NEXT block's W1."""
                pr1 = pp.tile([16, TB], F32, tag="sc3", bufs=3, name="pr1")
                for kc in range(KC):
                    te.matmul(pr1[0:1, :], C["onecb"][:],
                              x2[:, kc * TB:(kc + 1) * TB],
                              start=(kc == 0), stop=(kc == KC - 1))
                pr2 = pp.tile([16, TB], F32, tag="sc3", bufs=3, name="pr2")
                for kc in range(KC):
                    te.matmul(pr2[0:1, :], C["onecb"][:],
                              sq[:, kc * TB:(kc + 1) * TB],
                              start=(kc == 0), stop=(kc == KC - 1))
                psA = pp.tile([128, TB], F32, tag="red", bufs=1, name="psA")
                for c in range(3):
                    for kc in range(KC):
                        te.matmul(psA[32 * c:32 * c + 1, :],
                                  C["Wwt"][:, c * KC + kc:c * KC + kc + 1],
                                  x2[:, kc * TB:(kc + 1) * TB],
                                  tile_position=(0, 32 * c),
                                  start=(kc == 0), stop=(kc == KC - 1))
                return pr1, pr2, psA

            def emit_stats2(blk, pr1, pr2, psA):
                """Serial chain + softmax; no PE instructions at all."""
                t0 = blk * TB
                mub = wk.tile([1, TB], BF16, tag="f_mub", bufs=1, name="mub")[:]
                mu2 = wk.tile([1, TB], F32, tag="f_s", bufs=3, name="mu2")[:]
                var = wk.tile([1, TB], F32, tag="f_s", bufs=3, name="var")[:]
                sd = wk.tile([1, TB], F32, tag="f_s", bufs=3, name="sd")[:]
                rs = wk.tile([1, TB], F32, tag="f_s", bufs=3, name="rs")[:]
                nrb = wk.tile([1, TB], BF16, tag="f_nrb", bufs=1, name="nrb")[:]
                s.activation(mub, pr1[0:1, :], AF.Copy, scale=1.0 / DIM)
                s.activation(mu2, mub, AF.Square)
                v.scalar_tensor_tensor(var, pr2[0:1, :], 1.0 / DIM, mu2,
                                       ALU.mult, ALU.subtract)
                s.activation(sd, var, AF.Sqrt, bias=C["epsc"][:])
                v.reciprocal(rs, sd)
                s.activation(nrb, rs, AF.Copy, scale=-1.0)
                smu = wk.tile([65, TB], BF16, tag="f_smu", bufs=1, name="smu")
                gp.partition_broadcast(smu[:], mub, channels=65)
                snr = wk.tile([65, TB], BF16, tag="f_snr", bufs=1, name="snr")
                gp.partition_broadcast(snr[:], nrb, channels=65)
                zt = wk.tile([65, TB], F32, tag="zt", bufs=1, name="zt")
                for c in range(3):
                    r = slice(32 * c, 32 * c + 1)
                    v.scalar_tensor_tensor(zt[r, :], smu[r, :],
                                           C["ucol"][r, :], psA[r, :],
                                           ALU.mult, ALU.subtract)
                    v.tensor_mul(zt[r, :], zt[r, :], snr[r, :])
                    s.activation(et[r, :], zt[r, :], AF.Exp,
                                 bias=C["bpcol"][r, :])
                ssum = wk.tile([65, TB], F32, tag="ssum", bufs=1, name="ssum")
                gp.partition_all_reduce(ssum[:], et[:], channels=65,
                                        reduce_op=bass_isa.ReduceOp.add)
                rin = wk.tile([65, TB], F32, tag="rin", bufs=1, name="rin")
                v.reciprocal(rin[:], ssum[:])
                pot = wk.tile([65, TB], F32, tag="pot", bufs=1, name="pot")
                for c in range(3):
                    r = slice(32 * c, 32 * c + 1)
                    v.tensor_mul(pot[r, :], et[r, :], rin[r, :])
                    s.dma_start(out=io["outT"][c:c + 1, t0:t0 + TB],
                                in_=pot[r, :])

            # software pipeline over blocks
            xb_cur = emit_xb(0)
            pend = None                      # (blk, x2, sq) awaiting stats
            stat = None                      # (blk, pr1, pr2, psA)
            for blk in range(nblk):
                hb = emit_w1(xb_cur)
                if pend is not None:
                    stat = (pend[0],) + emit_stats1(pend[1], pend[2])
                xb_nxt = emit_xb(blk + 1) if blk + 1 < nblk else None
                x2, sq = emit_w2(xb_cur, hb)
                if stat is not None:
                    emit_stats2(*stat)
                    stat = None
                pend = (blk, x2, sq)
                xb_cur = xb_nxt
            st = emit_stats1(pend[1], pend[2])
            emit_stats2(pend[0], *st)


def build_program(tpc=TPC):
    nc = bacc.Bacc("TRN2", target_bir_lowering=False, debug=False)
    io = {}

    def din(name, shape, dtype, kind="ExternalInput"):
        io[name] = nc.dram_tensor(name, shape, dtype, kind=kind).ap()

    for name in ("m0", "m1", "m2", "dom"):
        din(name, [DIM, tpc], BF16)
    for name in ("Wqr", "Wgqr", "Wkr", "Wvr"):
        din(name, [DIM, DIM], BF16)
    din("W1r", [FFN, DIM], BF16)
    din("W2r", [DIM, FFN], BF16)
    din("Ssel", [128, 128], BF16)
    din("zer", [128, TB], BF16)
    din("Eexp", [16, 1024], BF16)
    din("onecb", [128, 1], BF16)
    for name, w in (("bq2", KC), ("bk", KC), ("bv", KC),
                    ("b1", MC1), ("b2", KC), ("g1", KC), ("be1", KC)):
        din(name, [128, w], F32)
    din("Wwt", [128, 3 * KC], BF16)
    din("ucol", [128, 1], F32)
    din("bpcol", [128, 1], F32)
    din("epsc", [1, 1], F32)
    io["xs"] = nc.dram_tensor("xs", [DIM, tpc], BF16).ap()
    io["outT"] = nc.dram_tensor("outT", [3, tpc], F32,
                                kind="ExternalOutput").ap()

    with tile.TileContext(nc) as tc:
        _emit(nc, tc, io, tpc)
    nc.compile()
    return nc


def _chunk_cols(vec, width):
    """[width*128] host vector -> [128, width] chunk-column layout."""
    return np.ascontiguousarray(vec.reshape(width, 128).T).astype(np.float32)


def _wr(Wf32, nkc, nmc):
    """[nkc*128, nmc*128] -> [nmc*128, nkc*128] per-mc stationary layout:
    out[mc*128+p, kc*128+j] = W[kc*128+p, mc*128+j]."""
    bf = ml_dtypes.bfloat16
    W = Wf32.reshape(nkc, 128, nmc, 128).transpose(2, 1, 0, 3)
    return np.ascontiguousarray(W.reshape(nmc * 128, nkc * 128)).astype(bf)


def prep_host_inputs(inputs, tpc=TPC, ncores=NCORES):
    """Per-core input maps (host-side shard + transpose + bf16 + folds)."""
    bf = ml_dtypes.bfloat16
    f32 = np.float32

    def fm(x):  # [B, DIM] -> [DIM, B] bf16 feature-major
        return np.ascontiguousarray(np.asarray(x, f32).T.astype(bf))

    m0 = fm(inputs["m0"]); m1 = fm(inputs["m1"]); m2 = fm(inputs["m2"])
    dom = fm(inputs["domain_rep"])

    head_of = np.arange(DIM) // HD
    S = np.zeros((128, 128), f32)
    E = np.zeros((16, 1024), f32)
    for c in range(KC):
        for p in range(128):
            h = head_of[c * 128 + p]
            S[p, c * 16 + h] = 1.0
            E[h, c * 128 + p] = 1.0

    Wg = np.asarray(inputs["Wg"], f32); Wq = np.asarray(inputs["Wq"], f32)
    sc = 1.0 / np.sqrt(HD)
    Wq_s = Wq * sc
    Wgq_s = (Wg @ Wq) * (sc / 3.0)
    bq2 = (np.asarray(inputs["bg"], f32) @ Wq
           + np.asarray(inputs["bq"], f32)) * sc

    # logits fold: W' = g2*Ww, u_c = colsum(bf16(W')), b'_c = beta2@Ww + bw
    g2 = np.asarray(inputs["g2"], f32)
    Ww = np.asarray(inputs["Ww"], f32)
    Wp = (g2[:, None] * Ww)
    u = Wp.astype(bf).astype(f32).sum(axis=0)              # [3]
    bp = np.asarray(inputs["beta2"], f32) @ Ww + np.asarray(inputs["bw"], f32)
    # Wwt[p, c*KC+kc] = W'[kc*128+p, c]
    Wwt = np.ascontiguousarray(
        Wp.reshape(KC, 128, 3).transpose(1, 2, 0).reshape(128, 3 * KC)
    ).astype(bf)
    ucol = np.zeros((128, 1), f32)
    bpcol = np.zeros((128, 1), f32)
    for c in range(3):
        ucol[32 * c, 0] = u[c]
        bpcol[32 * c, 0] = bp[c]

    consts = {
        "Wqr": _wr(Wq_s, KC, KC),
        "Wgqr": _wr(Wgq_s, KC, KC),
        "Wkr": _wr(np.asarray(inputs["Wk"], f32), KC, KC),
        "Wvr": _wr(np.asarray(inputs["Wv"], f32), KC, KC),
        "W1r": _wr(np.asarray(inputs["W1"], f32), KC, MC1),
        "W2r": _wr(np.asarray(inputs["W2"], f32), MC1, KC),
        "Ssel": S.astype(bf),
        "zer": np.zeros((128, TB), f32).astype(bf),
        "Eexp": E.astype(bf),
        "onecb": np.ones((128, 1), f32).astype(bf),
        "bq2": _chunk_cols(bq2, KC),
        "bk": _chunk_cols(np.asarray(inputs["bk"], f32), KC),
        "bv": _chunk_cols(np.asarray(inputs["bv"], f32), KC),
        "b1": _chunk_cols(np.asarray(inputs["b1"], f32), MC1),
        "b2": _chunk_cols(np.asarray(inputs["b2"], f32), KC),
        "g1": _chunk_cols(np.asarray(inputs["g1"], f32), KC),
        "be1": _chunk_cols(np.asarray(inputs["beta1"], f32), KC),
        "Wwt": Wwt,
        "ucol": ucol,
        "bpcol": bpcol,
        "epsc": np.full((1, 1), EPS, f32),
    }

    in_maps = []
    for c in range(ncores):
        sl = slice(c * tpc, (c + 1) * tpc)
        m = dict(consts)
        m["m0"] = np.ascontiguousarray(m0[:, sl])
        m["m1"] = np.ascontiguousarray(m1[:, sl])
        m["m2"] = np.ascontiguousarray(m2[:, sl])
        m["dom"] = np.ascontiguousarray(dom[:, sl])
        in_maps.append(m)
    return in_maps


def kernel(**inputs):
    from concourse.bass_utils import run_bass_kernel_spmd
    nc = build_program()
    in_maps = prep_host_inputs(inputs)
    res = run_bass_kernel_spmd(nc, in_maps, list(range(NCORES)))
    out = np.concatenate(
        [res.results[c]["outT"].T for c in range(NCORES)], axis=0)
    return np.ascontiguousarray(out.astype(np.float32))
